# revision 4
# baseline (speedup 1.0000x reference)
"""GTN (graph transformer network) meta-path kernel for TRN2, 8 NeuronCores.

Math (reference):
    Ap = A transposed to [E, N, N]
    a  = sum_e softmax(w1_0)[c,e] * Ap[e]      (per channel c)
    b  = sum_e softmax(w2_0)[c,e] * Ap[e]
    H  = a @ b
    twice:  H = normalize(H) @ gtconv(Ap, w)   (normalize = zero diag, col-scale)
    out = symmetrized mean over channels.

Sharding: channel-parallel — core c computes channel c end to end (the four
softmax mixes differ only in the tiny [E] weight vector, fed per-core), then
one AllReduce over the 8 cores and a local symmetrization.

On-device formulation works with TRANSPOSED intermediates Ht = H^T so that
 - every GEMM's moving operand is the previous GEMM's output as-is,
 - normalization becomes row sums (free-dim reduce) + per-partition scale.
Mixes are computed on the PE by putting (j,e) on partitions via bf16
DMA-transpose and multiplying with a block-diagonal weight matrix.
"""

import numpy as np

N = 2048
E = 8
C = 8
P = 128
NCORES = 8

_PROGRAM = None


def _softmax_rows(w: np.ndarray) -> np.ndarray:
    """w: [C, E, 1, 1] -> softmax over E, float64 precision, returns [C, E]."""
    x = w.reshape(C, E).astype(np.float64)
    x = x - x.max(axis=1, keepdims=True)
    ex = np.exp(x)
    return ex / ex.sum(axis=1, keepdims=True)


def _build_program():
    import concourse.bacc as bacc
    import concourse.bass as bass
    import concourse.mybir as mybir
    import concourse.tile as tile
    from concourse.masks import make_identity

    f32 = mybir.dt.float32
    bf16 = mybir.dt.bfloat16
    AX = mybir.AxisListType.X
    MUL = mybir.AluOpType.mult
    ADD = mybir.AluOpType.add
    NE = mybir.AluOpType.not_equal
    COPY = mybir.ActivationFunctionType.Copy

    nc = bacc.Bacc("TRN2")
    A_ext = nc.dram_tensor("A", [N, N, E], bf16, kind="ExternalInput")
    wblk_ext = nc.dram_tensor("wblk", [P, 64], bf16, kind="ExternalInput")
    out_ext = nc.dram_tensor("out", [N, N], f32, kind="ExternalOutput")

    with tile.TileContext(nc) as tc:
        with (
            tc.tile_pool(name="dram", bufs=1, space="DRAM") as dpool,
            tc.tile_pool(name="const", bufs=1) as cpool,
        ):
            # Mq[x, y] = sum_e w_q[e] * A[y, x, e]  (bf16, transposed mixes)
            # q order: 0=a, 1=b, 2=g1, 3=g2
            planes = [dpool.tile([N, N], bf16, name=f"plane{q}") for q in range(4)]
            h2t = dpool.tile([N, N], f32)           # per-channel H''^T
            s_sh = dpool.tile([N, N], f32, addr_space="Shared")  # allreduced

            # --- constants ---
            wblk_bf = cpool.tile([P, 64], bf16)
            nc.sync.dma_start(out=wblk_bf[:], in_=wblk_ext[:])
            ident = cpool.tile([P, P], f32)
            make_identity(nc, ident[:])
            # diag masks: masks[:, v, y] = 0 where y == p + v*128 else 1
            masks = cpool.tile([P, 4, 512], f32)
            nc.gpsimd.memset(masks[:], 1.0)
            for v in range(4):
                nc.gpsimd.affine_select(
                    out=masks[:, v],
                    in_=masks[:, v],
                    compare_op=NE,
                    fill=0.0,
                    base=v * P,
                    pattern=[[-1, 512]],
                    channel_multiplier=1,
                )

            # =========== Phase 1: mixes ===========
            # For each (256-k, 512-j) chunk: cast-load A natural, DMA-transpose
            # to put (16j x 8e) on partitions, matmul with block-diag weights.
            with (
                tc.tile_pool(name="mix", bufs=3) as mpool,
                tc.tile_pool(name="mixps", bufs=4, space="PSUM") as mpsum,
            ):
                plane_views = [
                    pl[:].rearrange(
                        "(jo bp h p) k -> jo h p bp k", jo=4, bp=16, h=2, p=16
                    )
                    for pl in planes
                ]
                for ko in range(8):  # 256 k rows per round
                    for jc in range(4):  # 512 j cols per round
                        ttr = mpool.tile([P, 2, 32, P], bf16, tag="ttr")
                        for kk in range(2):
                            k0 = ko * 256 + kk * P
                            nc.sync.dma_start(
                                out=ttr[:, kk],
                                in_=A_ext[k0 : k0 + P, jc * 512 : (jc + 1) * 512, :],
                                transpose=True,
                            )
                        stage = mpool.tile([P, 16, 256], bf16, tag="stage")
                        for bp in range(16):
                            pm = mpsum.tile([P, 256], f32, tag="pm")
                            for half in range(2):
                                blk = bp * 2 + half
                                nc.tensor.matmul(
                                    pm[half * 64 : (half + 1) * 64, :],
                                    lhsT=wblk_bf[:],
                                    rhs=ttr[:, :, blk, :],
                                    start=True,
                                    stop=True,
                                )
                            nc.vector.tensor_copy(out=stage[:, bp], in_=pm[:])
                        for q in range(4):
                            for half in range(2):
                                src = stage[
                                    half * 64 + q * 16 : half * 64 + (q + 1) * 16
                                ]
                                dst = plane_views[q][jc, half][
                                    :, :, ko * 256 : (ko + 1) * 256
                                ]
                                nc.sync.dma_start(out=dst, in_=src)

            # =========== Phases 2-4: three chained GEMMs ===========
            with (
                tc.tile_pool(name="big", bufs=1) as bigpool,
                tc.tile_pool(name="gw", bufs=4) as gpool,
                tc.tile_pool(name="nrm", bufs=4) as npool,
                tc.tile_pool(name="gps", bufs=2, space="PSUM") as gpsum,
            ):
                mv = [
                    bigpool.tile([P, 16, N], bf16, tag="mv0", name="mva"),
                    bigpool.tile([P, 16, N], bf16, tag="mv1", name="mvb"),
                ]
                # moving operand of GEMM1 = a^T rows (Ma), loaded once
                nc.sync.dma_start(
                    out=mv[0][:],
                    in_=planes[0][:].rearrange("(kc p) i -> p kc i", p=P),
                )

                def gemm(lhs_plane, rhs_res, out_res, normalize):
                    """out = lhs^T-mix @ rhs (transposed-chain step).

                    lhs_plane: DRAM Mq plane (transposed mix), naturalized
                        per-tile via DMA-transpose loads.
                    rhs_res:   SBUF-resident moving operand [P, 16, N] bf16.
                    out_res:   SBUF [P, 16, N] bf16 (normalize=True) or None
                        (normalize=False -> evict f32 to h2t).
                    """
                    for ms in range(16):
                        ps = [
                            gpsum.tile([P, 512], f32, tag=f"ps{ic}", name=f"ps{ic}")
                            for ic in range(4)
                        ]
                        for kc in range(16):
                            bt = gpool.tile([P, P], bf16, tag="bt")
                            nc.sync.dma_start(
                                out=bt[:],
                                in_=lhs_plane[
                                    ms * P : (ms + 1) * P, kc * P : (kc + 1) * P
                                ],
                                transpose=True,
                            )
                            for ic in range(4):
                                nc.tensor.matmul(
                                    ps[ic][:],
                                    lhsT=bt[:],
                                    rhs=rhs_res[:, kc, ic * 512 : (ic + 1) * 512],
                                    start=(kc == 0),
                                    stop=(kc == 15),
                                )
                        if normalize:
                            dc = (ms * P) // 512
                            v = ms % 4
                            degp = npool.tile([P, 4], f32, tag="degp")
                            # zero the diagonal in place + row-sum of masked tile
                            nc.vector.scalar_tensor_tensor(
                                out=ps[dc][:],
                                in0=ps[dc][:],
                                scalar=1.0,
                                in1=masks[:, v],
                                op0=MUL,
                                op1=MUL,
                                accum_out=degp[:, dc : dc + 1],
                            )
                            for ic in range(4):
                                if ic != dc:
                                    nc.vector.tensor_reduce(
                                        degp[:, ic : ic + 1], ps[ic][:], AX, ADD
                                    )
                            degs = npool.tile([P, 1], f32, tag="degs")
                            nc.vector.tensor_reduce(degs[:], degp[:], AX, ADD)
                            dinv = npool.tile([P, 1], f32, tag="dinv")
                            nc.vector.reciprocal(dinv[:], degs[:])
                            for ic in range(4):
                                nc.scalar.activation(
                                    out_res[:, ms, ic * 512 : (ic + 1) * 512],
                                    ps[ic][:],
                                    COPY,
                                    scale=dinv[:],
                                )
                        else:
                            for ic in range(4):
                                st = gpool.tile([P, 512], f32, tag="fstage")
                                nc.scalar.copy(st[:], ps[ic][:])
                                nc.sync.dma_start(
                                    out=h2t[
                                        ms * P : (ms + 1) * P,
                                        ic * 512 : (ic + 1) * 512,
                                    ],
                                    in_=st[:],
                                )

                # GEMM1: Ht = b^T a^T ; normalize -> Hnt in mv[1]
                gemm(planes[1], mv[0], mv[1], normalize=True)
                # GEMM2: H't = g1^T Hnt ; normalize -> H'nt (reuse mv0 slot)
                mv0b = bigpool.tile([P, 16, N], bf16, tag="mv0")
                gemm(planes[2], mv[1], mv0b, normalize=True)
                # GEMM3: H''t = g2^T H'nt -> h2t (f32), g2 pre-scaled by 1/16
                gemm(planes[3], mv0b, None, normalize=False)

            # =========== Phase 5: AllReduce over channels ===========
            nc.gpsimd.collective_compute(
                "AllReduce",
                ADD,
                replica_groups=[list(range(NCORES))],
                ins=[h2t.opt()],
                outs=[s_sh.opt()],
            )

            # =========== Phase 6: symmetrize out = S + S^T ===========
            with (
                tc.tile_pool(name="p6", bufs=3) as p6pool,
                tc.tile_pool(name="p6ps", bufs=4, space="PSUM") as p6psum,
            ):
                for ms in range(16):
                    srow = p6pool.tile([P, N], f32, tag="srow")
                    nc.sync.dma_start(
                        out=srow[:], in_=s_sh[ms * P : (ms + 1) * P, :]
                    )
                    ost = p6pool.tile([P, N], f32, tag="ost")
                    for nb in range(16):
                        scol = p6pool.tile([P, P], f32, tag="scol")
                        nc.sync.dma_start(
                            out=scol[:],
                            in_=s_sh[nb * P : (nb + 1) * P, ms * P : (ms + 1) * P],
                        )
                        pst = p6psum.tile([P, P], f32, tag="pst")
                        nc.tensor.transpose(pst[:], scol[:], ident[:])
                        nc.vector.scalar_tensor_tensor(
                            out=ost[:, nb * P : (nb + 1) * P],
                            in0=srow[:, nb * P : (nb + 1) * P],
                            scalar=1.0,
                            in1=pst[:],
                            op0=MUL,
                            op1=ADD,
                        )
                    nc.sync.dma_start(
                        out=out_ext[ms * P : (ms + 1) * P, :], in_=ost[:]
                    )

    nc.compile()
    return nc


def _get_program():
    global _PROGRAM
    if _PROGRAM is None:
        _PROGRAM = _build_program()
    return _PROGRAM


def _make_wblk(sws) -> np.ndarray:
    """Per-core block-diagonal mix weights.

    wblk[(j*8+e), (q*16+j)] = sws[q][e]  for j in 0..15.
    lhsT layout: partitions = (16 j, 8 e) matching the DMA-transposed A tile;
    out partitions = (4 q, 16 j).
    """
    wblk = np.zeros((P, 64), np.float32)
    for q, sw in enumerate(sws):
        for j in range(16):
            wblk[j * 8 : (j + 1) * 8, q * 16 + j] = sw.astype(np.float32)
    return wblk


def kernel(A, w1_0, w2_0, w_1, w_2):
    from concourse.bass_utils import run_bass_kernel_spmd

    swa = _softmax_rows(np.asarray(w1_0))
    swb = _softmax_rows(np.asarray(w2_0))
    sg1 = _softmax_rows(np.asarray(w_1))
    # fold mean over channels (1/8) and symmetrize (1/2) into the last mix
    sg2 = _softmax_rows(np.asarray(w_2)) / 16.0

    import ml_dtypes

    a_full = np.ascontiguousarray(
        np.asarray(A, dtype=np.float32)[0].astype(ml_dtypes.bfloat16)
    )  # [N,N,E] bf16
    in_maps = []
    for c in range(NCORES):
        wblk = _make_wblk([swa[c], swb[c], sg1[c], sg2[c]]).astype(ml_dtypes.bfloat16)
        in_maps.append({"A": a_full, "wblk": wblk})

    nc = _get_program()
    res = run_bass_kernel_spmd(nc, in_maps, list(range(NCORES)))
    return np.asarray(res.results[0]["out"], dtype=np.float32)
